# revision 28
# baseline (speedup 1.0000x reference)
"""Fused attention-block kernel for Trainium2, 8-core data-parallel over batch.

Computation (see harness reference): three BN+ReLU linear branches from the
same input, attention (QK^T/16 -> softmax -> AV), then a fourth BN+ReLU
linear.  BatchNorm1d is training-mode per-channel over (batch, feature) with
channel = sequence position, so batch-sharding needs a cross-core stats
all-reduce (sync-BN); weights are replicated.

v2 structure (vs v1 baseline):
 - x is cast+transposed on the HOST -> device does one contiguous 4MB load.
 - One tiny warmup AllReduce at t=0 absorbs cross-core launch skew.
 - z1/z2/z3 are separate matmul passes, each immediately followed by its own
   small stats AllReduce; later passes hide each AR's latency.
 - BN mean sums come from free tensor-column tricks / big fused DVE
   tensor_tensor_reduce ops instead of per-batch bn_stats.
 - Attention computes r^T directly (AV with x3 as lhsT), softmax row sums via
   tiny ones-matmuls, and folds 1/rowsum into the L4 psum->sbuf copy, so no
   extra transpose or normalize passes exist.
 - Element-wise work is balanced across ACT/DVE/Pool with pool ops placed
   only where they cannot overlap an in-flight collective.

Hardcoded: B=256, N=256, D=256, 8 cores -> 32 batches (8192 tokens) per core.
"""
import sys
import types

sys.path.insert(0, "/opt/trn_rl_repo")

import numpy as np
import ml_dtypes
from contextlib import ExitStack

import concourse.bass as bass
import concourse.mybir as mybir
import concourse.tile as tile
from concourse.masks import make_identity

BF16 = mybir.dt.bfloat16
F32 = mybir.dt.float32
NCORES = 8
B_LOC = 32          # batches per core
T = B_LOC * 256     # tokens per core
EPS = 1e-5
AL = mybir.AluOpType
ACT = mybir.ActivationFunctionType


def _install_profile_shim():
    """run_bass_kernel_spmd(trace=True) under axon needs antenv.axon_hooks,
    which this image lacks; synthesize it (harmless if tracing unused)."""
    if "antenv.axon_hooks" in sys.modules:
        return
    try:
        import antenv
        mod = types.ModuleType("antenv.axon_hooks")
        mod._hook = None
        mod.set_axon_ntff_profile_hook = lambda h: setattr(mod, "_hook", h)
        mod.get_axon_ntff_profile_hook = lambda: mod._hook
        sys.modules["antenv.axon_hooks"] = mod
        antenv.axon_hooks = mod
        from trn_agent_boot.trn_boot import _ntff_profile_via_ctypes
        hook = _ntff_profile_via_ctypes("/opt/axon/libaxon_pjrt.so")
        if hook is not None:
            mod.set_axon_ntff_profile_hook(hook)
    except Exception:
        pass


def _legalize_waits(nc, max_waits=1):
    """HW instructions carry one sync-wait slot; walrus rejects instructions
    with too many waits.  Hoist extras onto engine-matched NoOps."""
    for f in nc.m.functions:
        for bb in f.blocks:
            insts = bb.instructions
            new_list = []
            for inst in insts:
                si = inst.sync_info
                if si is not None and len(si.on_wait) > max_waits:
                    waits = list(si.on_wait)
                    extra, keep = waits[:-max_waits], waits[-max_waits:]
                    for j, w in enumerate(extra):
                        nop = mybir.InstNoOp(
                            name=f"{inst.name}-waitnop{j}",
                            engine=inst.engine,
                            ins=[], outs=[],
                            sync_info=mybir.SyncInfo(on_wait=[w], on_update=[]),
                        )
                        nc.register_instruction(nop, overwrite=True)
                        new_list.append(nop)
                    inst.sync_info = mybir.SyncInfo(
                        on_wait=keep, on_update=list(si.on_update))
                new_list.append(inst)
            del insts[:]
            for x in new_list:
                insts.append(x)


def build_program():
    nc = bass.Bass("TRN2", target_bir_lowering=False, debug=False,
                   num_devices=NCORES)

    xT_d = nc.dram_tensor("xT", [128, 2, T], BF16, kind="ExternalInput")
    w123_d = nc.dram_tensor("w123", [128, 2, 771], BF16, kind="ExternalInput")
    w4_d = nc.dram_tensor("w4", [128, 2, 258], BF16, kind="ExternalInput")
    bb_d = nc.dram_tensor("bb", [128, 4, 256], BF16, kind="ExternalInput")
    gb_d = nc.dram_tensor("gb", [128, 2, 2], F32, kind="ExternalInput")
    hc_d = nc.dram_tensor("hc", [128, 8], F32, kind="ExternalInput")
    out_d = nc.dram_tensor("out", [T, 256], BF16, kind="ExternalOutput")

    groups = [list(range(NCORES))]
    out_r = out_d.ap().rearrange("(b h p) e -> p b h e", b=B_LOC, h=2, p=128)

    with ExitStack() as ctx:
        tc = ctx.enter_context(tile.TileContext(nc))
        big = ctx.enter_context(tc.tile_pool(name="big", bufs=1))
        small = ctx.enter_context(tc.tile_pool(name="small", bufs=1))
        stage = ctx.enter_context(tc.tile_pool(name="stage", bufs=3))
        att = ctx.enter_context(tc.tile_pool(name="att", bufs=3))
        dram = ctx.enter_context(tc.tile_pool(name="dram", bufs=1, space="DRAM"))

        # ---- constants ------------------------------------------------------
        w123 = small.tile([128, 2, 771], BF16, tag="w123")
        w4 = small.tile([128, 2, 258], BF16, tag="w4")
        bbt = small.tile([128, 4, 256], BF16, tag="bbt")
        gbt = small.tile([128, 2, 2], F32, tag="gbt")
        hct = small.tile([128, 8], F32, tag="hct")
        idn = small.tile([128, 128], BF16, tag="idn")
        onesr = small.tile([1, 128], BF16, tag="onesr")
        nc.sync.dma_start(out=w123[:], in_=w123_d.ap())
        nc.sync.dma_start(out=w4[:], in_=w4_d.ap())
        nc.sync.dma_start(out=bbt[:], in_=bb_d.ap())
        nc.sync.dma_start(out=gbt[:], in_=gb_d.ap())
        nc.sync.dma_start(out=hct[:], in_=hc_d.ap())
        make_identity(nc, idn[:])
        nc.vector.memset(onesr[:], 1.0)

        # ---- warmup all-reduce: sync cores while input streams in ----------
        wu = small.tile([128, 1], F32, tag="wu")
        nc.vector.memset(wu[:], 0.0)
        wu_i = dram.tile([128, 1], F32, tag="wu_i")
        wu_o = dram.tile([128, 1], F32, tag="wu_o")
        nc.sync.dma_start(out=wu_i[:], in_=wu[:])
        nc.gpsimd.collective_compute(
            "AllReduce", AL.add, replica_groups=groups,
            ins=[wu_i[:].opt()], outs=[wu_o[:].opt()])

        # ---- xT load (8 chunks so z1 can start on chunk 0) ------------------
        xT = big.tile([128, 2, T], BF16, tag="tpX")
        NXC = 8
        for c in range(NXC):
            t0, t1 = c * (T // NXC), (c + 1) * (T // NXC)
            nc.sync.dma_start(out=xT[:, :, t0:t1],
                              in_=xT_d.ap()[:, :, t0:t1])

        # ---- big sbuf tiles -------------------------------------------------
        # z1sb carries 3 extra cols (256+l = sum_e z_l per token, from wsum
        # matmul columns).  x3a aliases z1sb; z4sb aliases z3sb.
        z1sb = big.tile([128, B_LOC, 2, 260], BF16, tag="tpA")
        z2sb = big.tile([128, B_LOC, 2, 256], BF16, tag="tpC")
        z3sb = big.tile([128, B_LOC, 2, 256], BF16, tag="tpB")
        x2T = big.tile([128, 2, T], BF16, tag="tpE")
        # per-engine square dumps (avoid cross-engine WAR serialization)
        scrD = big.tile([128, 8, 256], BF16, tag="scrD")
        scrA = big.tile([128, 8, 256], BF16, tag="scrA")
        scrP = big.tile([128, 8, 256], BF16, tag="scrP")

        # sum-of-squares accumulators per (layer, h, group)
        NG, GB = 4, 8
        qsum = small.tile([128, 3, 2, NG], F32, tag="qsum")

        def emit_allreduce(lidx, arin, width):
            ar_i = dram.tile([128, width], F32, tag=f"ari{lidx}", name=f"ai{lidx}")
            ar_o = dram.tile([128, width], F32, tag=f"aro{lidx}", name=f"ao{lidx}")
            nc.sync.dma_start(out=ar_i[:], in_=arin[:])
            nc.gpsimd.collective_compute(
                "AllReduce", AL.add, replica_groups=groups,
                ins=[ar_i[:].opt()], outs=[ar_o[:].opt()])
            artot = small.tile([128, width], F32, tag=f"artot{lidx}",
                               name=f"at{lidx}")
            nc.sync.dma_start(out=artot[:], in_=ar_o[:])
            return artot

        def pack_arin(lidx):
            """arin cols: (m_h0, q_h0, m_h1, q_h1) raw sums over (b, e)."""
            arin = small.tile([128, 4], F32, tag=f"arin{lidx}", name=f"an{lidx}")
            for h in range(2):
                nc.vector.tensor_reduce(out=arin[:, 2 * h:2 * h + 1],
                                        in_=z1sb[:, :, h, 256 + lidx:257 + lidx],
                                        axis=mybir.AxisListType.XY, op=AL.add)
                nc.vector.tensor_reduce(out=arin[:, 2 * h + 1:2 * h + 2],
                                        in_=qsum[:, lidx, h, :],
                                        axis=mybir.AxisListType.XY, op=AL.add)
            return arin

        # ---- z passes: layer l matmuls + copies + stats + AR ---------------
        zp_cm = tc.tile_pool(name="zp", bufs=4, space="PSUM")
        zp = zp_cm.__enter__()

        def z_pass(l):
            col0 = (0, 259, 515)[l]
            ncols = 259 if l == 0 else 256
            for b in range(B_LOC):
                psz = zp.tile([128, 2, 512], F32, tag="pz", name=f"pz{l}_{b}")
                for h in range(2):
                    for dc in range(2):
                        nc.tensor.matmul(
                            out=psz[:, h, 0:ncols],
                            lhsT=xT[:, dc, b * 256 + h * 128:b * 256 + (h + 1) * 128],
                            rhs=w123[:, dc, col0:col0 + ncols],
                            start=(dc == 0), stop=(dc == 1 and l != 2))
                    # z3 carries its bias (K=1 matmul) so the BN-apply can be
                    # a single per-partition op later.
                    if l == 2:
                        nc.tensor.matmul(out=psz[:, h, 0:256], lhsT=onesr[:],
                                         rhs=bbt[0:1, 2, :],
                                         start=False, stop=True)
                # psum -> sbuf copies (z1 on ACT, z2/z3 on DVE)
                if l == 0:
                    nc.scalar.copy(out=z1sb[:, b, :, 0:259], in_=psz[:, :, 0:259])
                elif l == 1:
                    nc.vector.tensor_scalar_add(z2sb[:, b, :, :],
                                                psz[:, :, 0:256], 0.0)
                else:
                    nc.vector.tensor_scalar_add(z3sb[:, b, :, :],
                                                psz[:, :, 0:256], 0.0)
                # grouped sum-of-squares when a batch-group completes:
                # z1 via DVE square+reduce, z2/z3 via one-pass ACT Square+accum
                if (b + 1) % GB == 0:
                    g = b // GB
                    gs = g * GB
                    src = (z1sb, z2sb, z3sb)[l]
                    for h in range(2):
                        zin = src[:, gs:gs + GB, h, 0:256]
                        if l == 0:
                            nc.vector.tensor_tensor(out=scrD[:], in0=zin,
                                                    in1=zin, op=AL.mult)
                            nc.vector.tensor_reduce(out=qsum[:, l, h, g:g + 1],
                                                    in_=scrD[:],
                                                    axis=mybir.AxisListType.XY,
                                                    op=AL.add)
                        else:
                            nc.scalar.activation(
                                out=scrA[:], in_=zin, func=ACT.Square,
                                accum_out=qsum[:, l, h, g:g + 1])
            return emit_allreduce(l, pack_arin(l), 4)

        artot1 = z_pass(0)
        artot2 = z_pass(1)
        artot3 = z_pass(2)
        zp_cm.__exit__(None, None, None)

        # ---- BN finalize: scale s + shifted-bias bst per layer --------------
        def bn_finalize(lidx, artot, wterm=None, exact_q=False, make_bst=True):
            """artot cols (m0,q0,m1,q1) = global raw sums over (b,e,cores).
            Returns (s fp32 [128,2], bst bf16 [128,2,256])."""
            norm = 1.0 / (NCORES * B_LOC * 256)
            meany = small.tile([128, 2], F32, tag=f"my{lidx}", name=f"my{lidx}")
            ey2 = small.tile([128, 2], F32, tag=f"ey{lidx}", name=f"ey{lidx}")
            nc.vector.tensor_scalar_mul(meany[:], artot[:, 0:4:2], norm)
            nc.vector.tensor_scalar_mul(ey2[:], artot[:, 1:4:2], norm)
            meanz = small.tile([128, 2], F32, tag=f"mz{lidx}", name=f"mz{lidx}")
            nc.vector.tensor_scalar_add(meanz[:], meany[:], hct[:, lidx:lidx + 1])
            varz = small.tile([128, 2], F32, tag=f"vz{lidx}", name=f"vz{lidx}")
            m2 = small.tile([128, 2], F32, tag=f"m2{lidx}", name=f"m2{lidx}")
            if exact_q:
                # q already measured E[(y+b)^2]; meanz includes mean(b)
                nc.vector.tensor_tensor(out=m2[:], in0=meanz[:], in1=meanz[:],
                                        op=AL.mult)
                nc.vector.tensor_tensor(out=varz[:], in0=ey2[:], in1=m2[:],
                                        op=AL.subtract)
                nc.vector.tensor_scalar_add(varz[:], varz[:], EPS)
            elif wterm is not None:
                # exact: E[z^2] = E[y^2] + 2 E[y b] + mean(b^2)
                eyb = small.tile([128, 2], F32, tag=f"eb{lidx}", name=f"eb{lidx}")
                nc.vector.tensor_scalar_mul(eyb[:], wterm[:], 2.0 * norm)
                nc.vector.tensor_tensor(out=ey2[:], in0=ey2[:], in1=eyb[:],
                                        op=AL.add)
                nc.vector.tensor_scalar_add(ey2[:], ey2[:], hct[:, 7:8])
                nc.vector.tensor_tensor(out=m2[:], in0=meanz[:], in1=meanz[:],
                                        op=AL.mult)
                nc.vector.tensor_tensor(out=varz[:], in0=ey2[:], in1=m2[:],
                                        op=AL.subtract)
                nc.vector.tensor_scalar_add(varz[:], varz[:], EPS)
            else:
                # var_z ~= var_y + var(b) (bias covariance negligible here)
                nc.vector.tensor_tensor(out=m2[:], in0=meany[:], in1=meany[:],
                                        op=AL.mult)
                nc.vector.tensor_tensor(out=varz[:], in0=ey2[:], in1=m2[:],
                                        op=AL.subtract)
                nc.vector.tensor_scalar(varz[:], varz[:],
                                        hct[:, 4 + lidx:5 + lidx], EPS,
                                        AL.add, AL.add)
            sd = small.tile([128, 2], F32, tag=f"sd{lidx}", name=f"sd{lidx}")
            nc.scalar.sqrt(out=sd[:], in_=varz[:])
            rstd = small.tile([128, 2], F32, tag=f"rs{lidx}", name=f"rs{lidx}")
            nc.vector.reciprocal(out=rstd[:], in_=sd[:])
            s = small.tile([128, 2], F32, tag=f"s{lidx}", name=f"s{lidx}")
            nc.vector.tensor_tensor(out=s[:], in0=rstd[:], in1=gbt[:, :, 0],
                                    op=AL.mult)
            tsh = small.tile([128, 2], F32, tag=f"t{lidx}", name=f"t{lidx}")
            nc.vector.tensor_tensor(out=tsh[:], in0=meanz[:], in1=s[:],
                                    op=AL.mult)
            nc.vector.tensor_tensor(out=tsh[:], in0=gbt[:, :, 1], in1=tsh[:],
                                    op=AL.subtract)
            if not make_bst:
                return s, tsh
            bst = small.tile([128, 2, 256], BF16, tag=f"b{lidx}", name=f"b{lidx}")
            for h in range(2):
                nc.vector.tensor_scalar(bst[:, h, :], bbt[:, lidx, :],
                                        s[:, h:h + 1], tsh[:, h:h + 1],
                                        AL.mult, AL.add)
            return s, bst

        # ---- transpose-affine: x1T/x2T = relu(z.T*s + bst.T) ----------------
        ap_cm = tc.tile_pool(name="ap", bufs=1, space="PSUM")
        ap = ap_cm.__enter__()
        x1T = big.tile([128, 2, T], BF16, tag="tpX")   # aliases xT
        dg = small.tile([128, 2, 2, 128], BF16, tag="dg")

        def t_pass(l, s_l, bst_l, xiT):
            zsb = (z1sb, z2sb)[l]
            for h in range(2):
                nc.vector.tensor_scalar_mul(dg[:, l, h, :], idn[:],
                                            s_l[:, h:h + 1])
            for b in range(B_LOC):
                pst = ap.tile([128, 2, 2, 128], F32, tag="pa", bufs=3,
                              name=f"pt{l}_{b}")
                for h in range(2):
                    for dc in range(2):
                        nc.tensor.matmul(
                            out=pst[:, dc, h, :],
                            lhsT=zsb[:, b, h, dc * 128:(dc + 1) * 128],
                            rhs=dg[:, l, h, :],
                            start=True, stop=False)
                        nc.tensor.matmul(
                            out=pst[:, dc, h, :],
                            lhsT=bst_l[:, h, dc * 128:(dc + 1) * 128],
                            rhs=idn[:],
                            start=False, stop=True)
                src = pst[:].rearrange("p dc h t -> p dc (h t)")
                if l == 0:
                    nc.scalar.activation(
                        out=xiT[:, :, b * 256:(b + 1) * 256], in_=src,
                        func=ACT.Relu)
                else:
                    nc.vector.tensor_scalar_max(
                        xiT[:, :, b * 256:(b + 1) * 256], src, 0.0)

        s1, bst1 = bn_finalize(0, artot1)
        t_pass(0, s1, bst1, x1T)
        s2, bst2 = bn_finalize(1, artot2)
        t_pass(1, s2, bst2, x2T)

        # ---- x3 path: s3 folded into exp bias (pt = s3[m]*exp(S/16)) and
        # ---- 1/s3 into the rowsum matmul; x3a = max(z3b + tsh3/s3, 0) is one
        # ---- pool op per (b, h), safe because it hard-depends on AR3.
        s3, tsh3 = bn_finalize(2, artot3, exact_q=True, make_bst=False)
        invs3 = small.tile([128, 2], F32, tag="invs3")
        nc.vector.reciprocal(out=invs3[:], in_=s3[:])
        invs3b = small.tile([128, 2], BF16, tag="invs3b")
        nc.vector.tensor_scalar_add(invs3b[:], invs3[:], 0.0)
        lns3 = small.tile([128, 2], F32, tag="lns3")
        nc.scalar.activation(out=lns3[:], in_=s3[:], func=ACT.Ln)
        tsh3s = small.tile([128, 2], F32, tag="tsh3s")
        nc.vector.tensor_tensor(out=tsh3s[:], in0=tsh3[:], in1=invs3[:],
                                op=AL.mult)
        x3a = big.tile([128, B_LOC, 2, 256], BF16, tag="tpA")

        def x3a_op(b):
            for h in range(2):
                nc.gpsimd.tensor_scalar(x3a[:, b, h, :], z3sb[:, b, h, :],
                                        tsh3s[:, h:h + 1], 0.0,
                                        AL.add, AL.max)

        # ---- attention + L4 --------------------------------------------------
        # z4sb: cols 0:256 = invr-scaled z4 (no bias), 256 = E[y*b4] col,
        # 257 = rowsum col (means); aliases z3 slot.
        z4sb = big.tile([128, B_LOC, 2, 258], BF16, tag="tpB")
        invrc = small.tile([128, B_LOC, 2], F32, tag="invrc")

        def l4_tail(b, psy0, psy1):
            nc.scalar.activation(out=z4sb[:, b, 0, :], in_=psy0[:],
                                 func=ACT.Copy, scale=invrc[:, b, 0:1])
            nc.vector.tensor_scalar_mul(z4sb[:, b, 1, :], psy1[:],
                                        invrc[:, b, 1:2])

        q4 = small.tile([128, 2, NG], F32, tag="q4")

        def z4_stats(g):
            gs = g * GB
            for h in range(2):
                zin = z4sb[:, gs:gs + GB, h, 0:256]
                nc.gpsimd.tensor_tensor(out=scrP[:], in0=zin, in1=zin,
                                        op=AL.mult)
                nc.vector.tensor_reduce(out=q4[:, h, g:g + 1], in_=scrP[:],
                                        axis=mybir.AxisListType.XY, op=AL.add)

        x3a_op(0)
        x3a_op(1)
        prev = None
        for b in range(B_LOC):
            if b + 2 < B_LOC:
                x3a_op(b + 2)
            if b % GB == 1 and b > GB:
                z4_stats(b // GB - 1)
            # S^T[m, n] = sum_e x2[m,e] x1[n,e]; exp via ACT (logits <= ~7)
            pss = ap.tile([128, 2, 256], F32, tag="pa", bufs=3, name=f"ps{b}")
            for mc in range(2):
                for ec in range(2):
                    nc.tensor.matmul(
                        out=pss[:, mc, :],
                        lhsT=x2T[:, ec, b * 256 + mc * 128:b * 256 + (mc + 1) * 128],
                        rhs=x1T[:, ec, b * 256:(b + 1) * 256],
                        start=(ec == 0), stop=(ec == 1))
            pt = att.tile([128, 2, 256], BF16, tag="pt", name=f"pt{b}")
            for mc in range(2):
                nc.scalar.activation(out=pt[:, mc, :], in_=pss[:, mc, :],
                                     scale=1.0 / 16.0, bias=lns3[:, mc:mc + 1],
                                     func=ACT.Exp)
            # r^T[d, n] directly: lhsT = x3 (token-major), rhs = P^T
            prt = ap.tile([128, 2, 256], F32, tag="prt", bufs=2, name=f"pr{b}")
            for dc in range(2):
                for mc in range(2):
                    nc.tensor.matmul(
                        out=prt[:, dc, :],
                        lhsT=x3a[:, b, mc, dc * 128:(dc + 1) * 128],
                        rhs=pt[:, mc, :],
                        start=(mc == 0), stop=(mc == 1))
            # softmax row sums as a column: sum_m exp = sum_m pt[m,n]/s3[m]
            pinv = ap.tile([128, 2], F32, tag="pinv", bufs=1, name=f"pi{b}")
            for nc_ in range(2):
                for mc in range(2):
                    nc.tensor.matmul(
                        out=pinv[:, nc_:nc_ + 1],
                        lhsT=pt[:, mc, nc_ * 128:(nc_ + 1) * 128],
                        rhs=invs3b[:, mc:mc + 1],
                        start=(mc == 0), stop=(mc == 1))
            nc.vector.reciprocal(out=invrc[:, b, :], in_=pinv[:])
            rT = att.tile([128, 2, 256], BF16, tag="rT", name=f"rT{b}")
            nc.scalar.copy(out=rT[:, 0, :], in_=prt[:, 0, :])
            nc.vector.tensor_scalar_add(rT[:, 1, :], prt[:, 1, :], 0.0)
            if prev is not None:
                l4_tail(*prev)
                prev = None
            # L4 for this batch (unnormalized; invr folded into psum copy)
            psy0 = ap.tile([128, 258], F32, tag="psy", bufs=2, name=f"py{b}_0")
            psy1 = ap.tile([128, 258], F32, tag="psy", bufs=2, name=f"py{b}_1")
            for h, psy in ((0, psy0), (1, psy1)):
                for dc in range(2):
                    nc.tensor.matmul(
                        out=psy[:, :],
                        lhsT=rT[:, dc, h * 128:(h + 1) * 128],
                        rhs=w4[:, dc, 0:258],
                        start=(dc == 0), stop=(dc == 1))
            prev = (b, psy0, psy1)
        l4_tail(*prev)

        # ---- L4 stats + AR4 -------------------------------------------------
        arin4 = small.tile([128, 6], F32, tag="arin4")
        z4_stats(NG - 1)
        for h in range(2):
            nc.vector.tensor_reduce(out=arin4[:, 2 * h + 1:2 * h + 2],
                                    in_=q4[:, h, :],
                                    axis=mybir.AxisListType.XY, op=AL.add)
            nc.vector.tensor_reduce(out=arin4[:, 2 * h:2 * h + 1],
                                    in_=z4sb[:, :, h, 257:258],
                                    axis=mybir.AxisListType.XY, op=AL.add)
            nc.vector.tensor_reduce(out=arin4[:, 4 + h:5 + h],
                                    in_=z4sb[:, :, h, 256:257],
                                    axis=mybir.AxisListType.XY, op=AL.add)
        artot4 = emit_allreduce(4, arin4, 6)
        s4, bst4 = bn_finalize(3, artot4, wterm=artot4[:, 4:6])
        # c4 = b4[e] + tsh4[n]/s4[n] for the pool path: out = s4*max(z+c4, 0)
        invs4 = small.tile([128, 2], F32, tag="invs4")
        nc.vector.reciprocal(out=invs4[:], in_=s4[:])
        c4 = small.tile([128, 2, 256], BF16, tag="c4")
        for h in range(2):
            nc.vector.tensor_scalar(c4[:, h, :], bst4[:, h, :],
                                    invs4[:, h:h + 1], 0.0, AL.mult, AL.add)

        # ---- final affine+relu split DVE+ACT / pool, then store -------------
        for b in range(B_LOC):
            ost = stage.tile([128, 2, 256], BF16, tag="ost", name=f"os{b}")
            orl = stage.tile([128, 2, 256], BF16, tag="orl", name=f"or{b}")
            if b % 8 < 5:
                for h in range(2):
                    nc.vector.scalar_tensor_tensor(
                        out=ost[:, h, :], in0=z4sb[:, b, h, 0:256],
                        scalar=s4[:, h:h + 1], in1=bst4[:, h, :],
                        op0=AL.mult, op1=AL.add)
                nc.scalar.activation(out=orl[:], in_=ost[:], func=ACT.Relu)
            else:
                nc.gpsimd.tensor_tensor(out=ost[:], in0=z4sb[:, b, :, 0:256],
                                        in1=c4[:], op=AL.add)
                for h in range(2):
                    nc.gpsimd.tensor_scalar(orl[:, h, :], ost[:, h, :],
                                            s4[:, h:h + 1], 0.0,
                                            AL.mult, AL.max)
            nc.sync.dma_start(out=out_r[:, b, :, :], in_=orl[:])

        ap_cm.__exit__(None, None, None)

    _legalize_waits(nc)
    return nc


_CACHE = {}


def _prep_core_inputs(inputs):
    bf = ml_dtypes.bfloat16
    W = [inputs["W1"], inputs["W2"], inputs["W3"], inputs["W4"]]
    bs = [inputs["b1"], inputs["b2"], inputs["b3"], inputs["b4"]]
    gamma, beta = inputs["gamma"], inputs["beta"]

    # w123 cols: [0:256]=W1^T | 256+l = wsum_l (sum_e W_l) | [259:515]=W2^T
    # | [515:771]=W3^T
    w123 = np.zeros((128, 2, 771), dtype=bf)
    col0 = (0, 259, 515)
    for c in range(2):
        for l in range(3):
            w123[:, c, col0[l]:col0[l] + 256] = \
                W[l][:, c * 128:(c + 1) * 128].T.astype(bf)
            ws = W[l].astype(np.float64).sum(axis=0).astype(np.float32)
            w123[:, c, 256 + l] = ws[c * 128:(c + 1) * 128].astype(bf)
    w4 = np.zeros((128, 2, 258), dtype=bf)
    wb4 = (W[3].T.astype(np.float64) @ bs[3].astype(np.float64)).astype(np.float32)
    ws4 = W[3].astype(np.float64).sum(axis=0).astype(np.float32)
    for c in range(2):
        w4[:, c, 0:256] = W[3][:, c * 128:(c + 1) * 128].T.astype(bf)
        w4[:, c, 256] = wb4[c * 128:(c + 1) * 128].astype(bf)
        w4[:, c, 257] = ws4[c * 128:(c + 1) * 128].astype(bf)
    bb = np.broadcast_to(np.stack(bs, 0)[None], (128, 4, 256)).astype(bf)
    bb = np.ascontiguousarray(bb)
    gb = np.zeros((128, 2, 2), dtype=np.float32)
    for h in range(2):
        gb[:, h, 0] = gamma[h * 128:(h + 1) * 128]
        gb[:, h, 1] = beta[h * 128:(h + 1) * 128]
    hc = np.zeros((128, 8), dtype=np.float32)
    for l in range(4):
        hc[:, l] = bs[l].mean(dtype=np.float64)
    for l in range(3):
        hc[:, 4 + l] = (bs[l].astype(np.float64) ** 2).mean() - \
            bs[l].mean(dtype=np.float64) ** 2
    hc[:, 7] = (bs[3].astype(np.float64) ** 2).mean()
    return w123, w4, bb, gb, hc


def kernel(**inputs):
    _install_profile_shim()
    from concourse.bass_utils import run_bass_kernel_spmd

    if "nc" not in _CACHE:
        _CACHE["nc"] = build_program()
    nc = _CACHE["nc"]

    x = np.asarray(inputs["x"], dtype=np.float32)
    w123, w4, bb, gb, hc = _prep_core_inputs(
        {k: np.asarray(v) for k, v in inputs.items()})

    bf = ml_dtypes.bfloat16
    in_maps = []
    for i in range(NCORES):
        xs = x[i * B_LOC:(i + 1) * B_LOC].reshape(T, 256)
        # xT[p, dc, t] = xs[t, dc*128 + p]
        xTh = np.ascontiguousarray(
            xs.T.reshape(2, 128, T).transpose(1, 0, 2)).astype(bf)
        in_maps.append({"xT": xTh, "w123": w123, "w4": w4, "bb": bb,
                        "gb": gb, "hc": hc})

    trace = _CACHE.get("trace", False)
    res = run_bass_kernel_spmd(nc, in_maps, list(range(NCORES)), trace=trace)
    _CACHE["last_result"] = res

    out = np.empty((256, 256, 256), dtype=np.float32)
    for i in range(NCORES):
        out[i * B_LOC:(i + 1) * B_LOC] = np.asarray(
            res.results[i]["out"], dtype=np.float32).reshape(B_LOC, 256, 256)
    return out


# revision 39
# speedup vs baseline: 1.7282x; 1.7282x over previous
"""Fused attention-block kernel for Trainium2, 8-core data-parallel over batch.

Computation (see harness reference): three BN+ReLU linear branches from the
same input, attention (QK^T/16 -> softmax -> AV), then a fourth BN+ReLU
linear.  BatchNorm1d is training-mode per-channel over (batch, feature) with
channel = sequence position, so batch-sharding needs a cross-core stats
all-reduce (sync-BN); weights are replicated.

v2 structure (vs v1 baseline):
 - x is cast+transposed on the HOST -> device does one contiguous 4MB load.
 - One tiny warmup AllReduce at t=0 absorbs cross-core launch skew.
 - z1/z2/z3 are separate matmul passes, each immediately followed by its own
   small stats AllReduce; later passes hide each AR's latency.
 - BN mean sums come from free tensor-column tricks / big fused DVE
   tensor_tensor_reduce ops instead of per-batch bn_stats.
 - Attention computes r^T directly (AV with x3 as lhsT), softmax row sums via
   tiny ones-matmuls, and folds 1/rowsum into the L4 psum->sbuf copy, so no
   extra transpose or normalize passes exist.
 - Element-wise work is balanced across ACT/DVE/Pool with pool ops placed
   only where they cannot overlap an in-flight collective.

Hardcoded: B=256, N=256, D=256, 8 cores -> 32 batches (8192 tokens) per core.
"""
import sys
import types

sys.path.insert(0, "/opt/trn_rl_repo")

import numpy as np
import ml_dtypes
from contextlib import ExitStack

import concourse.bass as bass
import concourse.mybir as mybir
import concourse.tile as tile
from concourse.masks import make_identity

BF16 = mybir.dt.bfloat16
F32 = mybir.dt.float32
NCORES = 8
B_LOC = 32          # batches per core
T = B_LOC * 256     # tokens per core
EPS = 1e-5
AL = mybir.AluOpType
ACT = mybir.ActivationFunctionType


def _install_profile_shim():
    """run_bass_kernel_spmd(trace=True) under axon needs antenv.axon_hooks,
    which this image lacks; synthesize it (harmless if tracing unused)."""
    if "antenv.axon_hooks" in sys.modules:
        return
    try:
        import antenv
        mod = types.ModuleType("antenv.axon_hooks")
        mod._hook = None
        mod.set_axon_ntff_profile_hook = lambda h: setattr(mod, "_hook", h)
        mod.get_axon_ntff_profile_hook = lambda: mod._hook
        sys.modules["antenv.axon_hooks"] = mod
        antenv.axon_hooks = mod
        from trn_agent_boot.trn_boot import _ntff_profile_via_ctypes
        hook = _ntff_profile_via_ctypes("/opt/axon/libaxon_pjrt.so")
        if hook is not None:
            mod.set_axon_ntff_profile_hook(hook)
    except Exception:
        pass


def _legalize_waits(nc, max_waits=1):
    """HW instructions carry one sync-wait slot; walrus rejects instructions
    with too many waits.  Hoist extras onto engine-matched NoOps."""
    for f in nc.m.functions:
        for bb in f.blocks:
            insts = bb.instructions
            new_list = []
            for inst in insts:
                si = inst.sync_info
                if si is not None and len(si.on_wait) > max_waits:
                    waits = list(si.on_wait)
                    extra, keep = waits[:-max_waits], waits[-max_waits:]
                    for j, w in enumerate(extra):
                        nop = mybir.InstNoOp(
                            name=f"{inst.name}-waitnop{j}",
                            engine=inst.engine,
                            ins=[], outs=[],
                            sync_info=mybir.SyncInfo(on_wait=[w], on_update=[]),
                        )
                        nc.register_instruction(nop, overwrite=True)
                        new_list.append(nop)
                    inst.sync_info = mybir.SyncInfo(
                        on_wait=keep, on_update=list(si.on_update))
                new_list.append(inst)
            del insts[:]
            for x in new_list:
                insts.append(x)


def build_program():
    nc = bass.Bass("TRN2", target_bir_lowering=False, debug=False,
                   num_devices=NCORES)

    xT_d = nc.dram_tensor("xT", [128, 2, T], BF16, kind="ExternalInput")
    w123_d = nc.dram_tensor("w123", [128, 2, 771], BF16, kind="ExternalInput")
    w4_d = nc.dram_tensor("w4", [128, 2, 257], BF16, kind="ExternalInput")
    bb_d = nc.dram_tensor("bb", [128, 4, 256], BF16, kind="ExternalInput")
    gb_d = nc.dram_tensor("gb", [128, 2, 2], F32, kind="ExternalInput")
    hc_d = nc.dram_tensor("hc", [128, 8], F32, kind="ExternalInput")
    out_d = nc.dram_tensor("out", [T, 256], BF16, kind="ExternalOutput")

    groups = [list(range(NCORES))]
    out_r = out_d.ap().rearrange("(b h p) e -> p b h e", b=B_LOC, h=2, p=128)

    with ExitStack() as ctx:
        tc = ctx.enter_context(tile.TileContext(nc))
        big = ctx.enter_context(tc.tile_pool(name="big", bufs=1))
        small = ctx.enter_context(tc.tile_pool(name="small", bufs=1))
        stage = ctx.enter_context(tc.tile_pool(name="stage", bufs=3))
        att = ctx.enter_context(tc.tile_pool(name="att", bufs=3))
        dram = ctx.enter_context(tc.tile_pool(name="dram", bufs=1, space="DRAM"))

        # ---- constants ------------------------------------------------------
        w123 = small.tile([128, 2, 771], BF16, tag="w123")
        w4 = small.tile([128, 2, 257], BF16, tag="w4")
        bbt = small.tile([128, 4, 256], BF16, tag="bbt")
        gbt = small.tile([128, 2, 2], F32, tag="gbt")
        hct = small.tile([128, 8], F32, tag="hct")
        idn = small.tile([128, 128], BF16, tag="idn")
        onesr = small.tile([1, 128], BF16, tag="onesr")
        onesc = small.tile([128, 1], BF16, tag="onesc")
        b4e = small.tile([128, 257], BF16, tag="b4e")
        nc.sync.dma_start(out=w123[:], in_=w123_d.ap())
        nc.sync.dma_start(out=w4[:], in_=w4_d.ap())
        nc.sync.dma_start(out=bbt[:], in_=bb_d.ap())
        nc.sync.dma_start(out=gbt[:], in_=gb_d.ap())
        nc.sync.dma_start(out=hct[:], in_=hc_d.ap())
        make_identity(nc, idn[:])
        nc.vector.memset(onesr[:], 1.0)
        nc.vector.memset(onesc[:], 1.0)
        nc.vector.memset(b4e[:, 256:257], 0.0)
        nc.vector.tensor_scalar_add(b4e[:, 0:256], bbt[:, 3, :], 0.0)

        # ---- warmup all-reduce: sync cores while input streams in ----------
        wu = small.tile([128, 1], F32, tag="wu")
        nc.vector.memset(wu[:], 0.0)
        wu_i = dram.tile([128, 1], F32, tag="wu_i")
        wu_o = dram.tile([128, 1], F32, tag="wu_o")
        nc.sync.dma_start(out=wu_i[:], in_=wu[:])
        nc.gpsimd.collective_compute(
            "AllReduce", AL.add, replica_groups=groups,
            ins=[wu_i[:].opt()], outs=[wu_o[:].opt()])

        # ---- xT load (8 chunks so z1 can start on chunk 0) ------------------
        xT = big.tile([128, 2, T], BF16, tag="tpX")
        NXC = 8
        for c in range(NXC):
            t0, t1 = c * (T // NXC), (c + 1) * (T // NXC)
            nc.sync.dma_start(out=xT[:, :, t0:t1],
                              in_=xT_d.ap()[:, :, t0:t1])

        # ---- big sbuf tiles -------------------------------------------------
        # z1sb carries 3 extra cols (256+l = sum_e z_l per token, from wsum
        # matmul columns).  x3a aliases z1sb; z4sb aliases z3sb.
        z1sb = big.tile([128, B_LOC, 2, 260], BF16, tag="tpA")
        z2sb = big.tile([128, B_LOC, 2, 256], BF16, tag="tpC")
        z3sb = big.tile([128, B_LOC, 2, 256], BF16, tag="tpB")
        x2T = big.tile([128, 2, T], BF16, tag="tpE")
        # per-engine square dumps (avoid cross-engine WAR serialization)
        scrD = big.tile([128, 8, 256], BF16, tag="scrD")
        scrA = big.tile([128, 8, 256], BF16, tag="scrA")

        # sum-of-squares accumulators per (layer, h, group)
        NG, GB = 4, 8
        qsum = small.tile([128, 3, 2, NG], F32, tag="qsum")

        def emit_allreduce(lidx, arin, width):
            ar_i = dram.tile([128, width], F32, tag=f"ari{lidx}", name=f"ai{lidx}")
            ar_o = dram.tile([128, width], F32, tag=f"aro{lidx}", name=f"ao{lidx}")
            nc.sync.dma_start(out=ar_i[:], in_=arin[:])
            nc.gpsimd.collective_compute(
                "AllReduce", AL.add, replica_groups=groups,
                ins=[ar_i[:].opt()], outs=[ar_o[:].opt()])
            artot = small.tile([128, width], F32, tag=f"artot{lidx}",
                               name=f"at{lidx}")
            nc.sync.dma_start(out=artot[:], in_=ar_o[:])
            return artot

        def pack_arin(lidx):
            """arin cols: (m_h0, q_h0, m_h1, q_h1) raw sums over (b, e)."""
            arin = small.tile([128, 4], F32, tag=f"arin{lidx}", name=f"an{lidx}")
            for h in range(2):
                nc.vector.tensor_reduce(out=arin[:, 2 * h:2 * h + 1],
                                        in_=z1sb[:, :, h, 256 + lidx:257 + lidx],
                                        axis=mybir.AxisListType.XY, op=AL.add)
                nc.vector.tensor_reduce(out=arin[:, 2 * h + 1:2 * h + 2],
                                        in_=qsum[:, lidx, h, :],
                                        axis=mybir.AxisListType.XY, op=AL.add)
            return arin

        # ---- z passes: layer l matmuls + copies + stats + AR ---------------
        zp_cm = tc.tile_pool(name="zp", bufs=4, space="PSUM")
        zp = zp_cm.__enter__()

        def z_pass(l):
            col0 = (0, 259, 515)[l]
            ncols = 259 if l == 0 else 256
            for b in range(B_LOC):
                psz = zp.tile([128, 2, 512], F32, tag="pz", name=f"pz{l}_{b}")
                for h in range(2):
                    for dc in range(2):
                        nc.tensor.matmul(
                            out=psz[:, h, 0:ncols],
                            lhsT=xT[:, dc, b * 256 + h * 128:b * 256 + (h + 1) * 128],
                            rhs=w123[:, dc, col0:col0 + ncols],
                            start=(dc == 0), stop=(dc == 1 and l != 2))
                    # z3 carries its bias (K=1 matmul) so the BN-apply can be
                    # a single per-partition op later.
                    if l == 2:
                        nc.tensor.matmul(out=psz[:, h, 0:256], lhsT=onesr[:],
                                         rhs=bbt[0:1, 2, :],
                                         start=False, stop=True)
                # psum -> sbuf copies (z1 on ACT, z2/z3 on DVE)
                if l == 0:
                    nc.scalar.copy(out=z1sb[:, b, :, 0:259], in_=psz[:, :, 0:259])
                elif l == 1:
                    nc.vector.tensor_scalar_add(z2sb[:, b, :, :],
                                                psz[:, :, 0:256], 0.0)
                else:
                    nc.vector.tensor_scalar_add(z3sb[:, b, :, :],
                                                psz[:, :, 0:256], 0.0)
                # grouped sum-of-squares when a batch-group completes:
                # z1 via DVE square+reduce, z2/z3 via one-pass ACT Square+accum
                if (b + 1) % GB == 0:
                    g = b // GB
                    gs = g * GB
                    src = (z1sb, z2sb, z3sb)[l]
                    for h in range(2):
                        zin = src[:, gs:gs + GB, h, 0:256]
                        if l == 0:
                            nc.vector.tensor_tensor(out=scrD[:], in0=zin,
                                                    in1=zin, op=AL.mult)
                            nc.vector.tensor_reduce(out=qsum[:, l, h, g:g + 1],
                                                    in_=scrD[:],
                                                    axis=mybir.AxisListType.XY,
                                                    op=AL.add)
                        else:
                            nc.scalar.activation(
                                out=scrA[:], in_=zin, func=ACT.Square,
                                accum_out=qsum[:, l, h, g:g + 1])
            return emit_allreduce(l, pack_arin(l), 4)

        artot1 = z_pass(0)
        artot2 = z_pass(1)
        artot3 = z_pass(2)
        zp_cm.__exit__(None, None, None)

        # ---- BN finalize: scale s + shifted-bias bst per layer --------------
        def bn_finalize(lidx, artot, wterm=None, exact_q=False, make_bst=True):
            """artot cols (m0,q0,m1,q1) = global raw sums over (b,e,cores).
            Returns (s fp32 [128,2], bst bf16 [128,2,256])."""
            norm = 1.0 / (NCORES * B_LOC * 256)
            meany = small.tile([128, 2], F32, tag=f"my{lidx}", name=f"my{lidx}")
            ey2 = small.tile([128, 2], F32, tag=f"ey{lidx}", name=f"ey{lidx}")
            nc.vector.tensor_scalar_mul(meany[:], artot[:, 0:4:2], norm)
            nc.vector.tensor_scalar_mul(ey2[:], artot[:, 1:4:2], norm)
            meanz = small.tile([128, 2], F32, tag=f"mz{lidx}", name=f"mz{lidx}")
            nc.vector.tensor_scalar_add(meanz[:], meany[:], hct[:, lidx:lidx + 1])
            varz = small.tile([128, 2], F32, tag=f"vz{lidx}", name=f"vz{lidx}")
            m2 = small.tile([128, 2], F32, tag=f"m2{lidx}", name=f"m2{lidx}")
            if exact_q:
                # q already measured E[(y+b)^2]; meanz includes mean(b)
                nc.vector.tensor_tensor(out=m2[:], in0=meanz[:], in1=meanz[:],
                                        op=AL.mult)
                nc.vector.tensor_tensor(out=varz[:], in0=ey2[:], in1=m2[:],
                                        op=AL.subtract)
                nc.vector.tensor_scalar_add(varz[:], varz[:], EPS)
            elif wterm is not None:
                # exact: E[z^2] = E[y^2] + 2 E[y b] + mean(b^2)
                eyb = small.tile([128, 2], F32, tag=f"eb{lidx}", name=f"eb{lidx}")
                nc.vector.tensor_scalar_mul(eyb[:], wterm[:], 2.0 * norm)
                nc.vector.tensor_tensor(out=ey2[:], in0=ey2[:], in1=eyb[:],
                                        op=AL.add)
                nc.vector.tensor_scalar_add(ey2[:], ey2[:], hct[:, 7:8])
                nc.vector.tensor_tensor(out=m2[:], in0=meanz[:], in1=meanz[:],
                                        op=AL.mult)
                nc.vector.tensor_tensor(out=varz[:], in0=ey2[:], in1=m2[:],
                                        op=AL.subtract)
                nc.vector.tensor_scalar_add(varz[:], varz[:], EPS)
            else:
                # var_z ~= var_y + var(b) (bias covariance negligible here)
                nc.vector.tensor_tensor(out=m2[:], in0=meany[:], in1=meany[:],
                                        op=AL.mult)
                nc.vector.tensor_tensor(out=varz[:], in0=ey2[:], in1=m2[:],
                                        op=AL.subtract)
                nc.vector.tensor_scalar(varz[:], varz[:],
                                        hct[:, 4 + lidx:5 + lidx], EPS,
                                        AL.add, AL.add)
            sd = small.tile([128, 2], F32, tag=f"sd{lidx}", name=f"sd{lidx}")
            nc.scalar.sqrt(out=sd[:], in_=varz[:])
            rstd = small.tile([128, 2], F32, tag=f"rs{lidx}", name=f"rs{lidx}")
            nc.vector.reciprocal(out=rstd[:], in_=sd[:])
            s = small.tile([128, 2], F32, tag=f"s{lidx}", name=f"s{lidx}")
            nc.vector.tensor_tensor(out=s[:], in0=rstd[:], in1=gbt[:, :, 0],
                                    op=AL.mult)
            tsh = small.tile([128, 2], F32, tag=f"t{lidx}", name=f"t{lidx}")
            nc.vector.tensor_tensor(out=tsh[:], in0=meanz[:], in1=s[:],
                                    op=AL.mult)
            nc.vector.tensor_tensor(out=tsh[:], in0=gbt[:, :, 1], in1=tsh[:],
                                    op=AL.subtract)
            if not make_bst:
                return s, tsh
            bst = small.tile([128, 2, 256], BF16, tag=f"b{lidx}", name=f"b{lidx}")
            for h in range(2):
                nc.vector.tensor_scalar(bst[:, h, :], bbt[:, lidx, :],
                                        s[:, h:h + 1], tsh[:, h:h + 1],
                                        AL.mult, AL.add)
            return s, bst

        # ---- transpose-affine: x1T/x2T = relu(z.T*s + bst.T) ----------------
        ap_cm = tc.tile_pool(name="ap", bufs=1, space="PSUM")
        ap = ap_cm.__enter__()
        x1T = big.tile([128, 2, T], BF16, tag="tpX")   # aliases xT
        dg = small.tile([128, 2, 2, 128], BF16, tag="dg")

        def t_pass(l, s_l, bst_l, xiT):
            zsb = (z1sb, z2sb)[l]
            for h in range(2):
                nc.vector.tensor_scalar_mul(dg[:, l, h, :], idn[:],
                                            s_l[:, h:h + 1])
            for b in range(B_LOC):
                pst = ap.tile([128, 2, 2, 128], F32, tag="pa", bufs=3,
                              name=f"pt{l}_{b}")
                for h in range(2):
                    for dc in range(2):
                        nc.tensor.matmul(
                            out=pst[:, dc, h, :],
                            lhsT=zsb[:, b, h, dc * 128:(dc + 1) * 128],
                            rhs=dg[:, l, h, :],
                            start=True, stop=False)
                        nc.tensor.matmul(
                            out=pst[:, dc, h, :],
                            lhsT=bst_l[:, h, dc * 128:(dc + 1) * 128],
                            rhs=idn[:],
                            start=False, stop=True)
                src = pst[:].rearrange("p dc h t -> p dc (h t)")
                if l == 0:
                    nc.scalar.activation(
                        out=xiT[:, :, b * 256:(b + 1) * 256], in_=src,
                        func=ACT.Relu)
                else:
                    nc.vector.tensor_scalar_max(
                        xiT[:, :, b * 256:(b + 1) * 256], src, 0.0)

        s1, bst1 = bn_finalize(0, artot1)
        t_pass(0, s1, bst1, x1T)
        s2, bst2 = bn_finalize(1, artot2)
        t_pass(1, s2, bst2, x2T)

        # ---- x3 = relu(s3*z3b + tsh3): z3b carries its bias, so this is a
        # ---- single ACT op per (b, h) with per-partition scale+bias.
        s3, tsh3 = bn_finalize(2, artot3, exact_q=True, make_bst=False)
        x3a = big.tile([128, B_LOC, 2, 256], BF16, tag="tpA")

        def x3a_op(b):
            for h in range(2):
                nc.scalar.activation(out=x3a[:, b, h, :], in_=z3sb[:, b, h, :],
                                     func=ACT.Relu, scale=s3[:, h:h + 1],
                                     bias=tsh3[:, h:h + 1])

        # ---- attention + L4 --------------------------------------------------
        # z4sb: cols 0:256 = invr-scaled z4 PLUS bias b4 (so the final BN
        # apply is one ACT op and the stats are exact), 256 = mean col.
        z4sb = big.tile([128, B_LOC, 2, 257], BF16, tag="tpB")
        invrc = small.tile([128, B_LOC, 2], F32, tag="invrc")

        def l4_tail(b, psy0, psy1):
            for h, psy in ((0, psy0), (1, psy1)):
                nc.vector.scalar_tensor_tensor(
                    out=z4sb[:, b, h, :], in0=psy[:],
                    scalar=invrc[:, b, h:h + 1], in1=b4e[:],
                    op0=AL.mult, op1=AL.add)

        q4 = small.tile([128, 2, NG], F32, tag="q4")

        def z4_stats(g):
            gs = g * GB
            for h in range(2):
                zin = z4sb[:, gs:gs + GB, h, 0:256]
                nc.vector.tensor_tensor(out=scrD[:], in0=zin, in1=zin,
                                        op=AL.mult)
                nc.vector.tensor_reduce(out=q4[:, h, g:g + 1], in_=scrD[:],
                                        axis=mybir.AxisListType.XY, op=AL.add)

        x3a_op(0)
        x3a_op(1)
        prev = None
        for b in range(B_LOC):
            if b + 2 < B_LOC:
                x3a_op(b + 2)
            if b % GB == 1 and b > GB:
                z4_stats(b // GB - 1)
            # S^T[m, n] = sum_e x2[m,e] x1[n,e]; exp via ACT (logits <= ~7)
            pss = ap.tile([128, 2, 256], F32, tag="pa", bufs=3, name=f"ps{b}")
            for mc in range(2):
                for ec in range(2):
                    nc.tensor.matmul(
                        out=pss[:, mc, :],
                        lhsT=x2T[:, ec, b * 256 + mc * 128:b * 256 + (mc + 1) * 128],
                        rhs=x1T[:, ec, b * 256:(b + 1) * 256],
                        start=(ec == 0), stop=(ec == 1))
            pt = att.tile([128, 2, 256], BF16, tag="pt", name=f"pt{b}")
            nc.scalar.activation(out=pt[:], in_=pss[:], scale=1.0 / 16.0,
                                 func=ACT.Exp)
            # r^T[d, n] directly: lhsT = x3 (token-major), rhs = P^T
            prt = ap.tile([128, 2, 256], F32, tag="prt", bufs=2, name=f"pr{b}")
            for dc in range(2):
                for mc in range(2):
                    nc.tensor.matmul(
                        out=prt[:, dc, :],
                        lhsT=x3a[:, b, mc, dc * 128:(dc + 1) * 128],
                        rhs=pt[:, mc, :],
                        start=(mc == 0), stop=(mc == 1))
            # softmax row sums as a column: sum_m P^T[m, n]
            pinv = ap.tile([128, 2], F32, tag="pinv", bufs=1, name=f"pi{b}")
            for nc_ in range(2):
                for mc in range(2):
                    nc.tensor.matmul(
                        out=pinv[:, nc_:nc_ + 1],
                        lhsT=pt[:, mc, nc_ * 128:(nc_ + 1) * 128],
                        rhs=onesc[:, 0:1],
                        start=(mc == 0), stop=(mc == 1))
            nc.vector.reciprocal(out=invrc[:, b, :], in_=pinv[:])
            rT = att.tile([128, 2, 256], BF16, tag="rT", name=f"rT{b}")
            nc.scalar.copy(out=rT[:, 0, :], in_=prt[:, 0, :])
            nc.vector.tensor_scalar_add(rT[:, 1, :], prt[:, 1, :], 0.0)
            if prev is not None:
                l4_tail(*prev)
                prev = None
            # L4 for this batch (unnormalized; invr + b4 folded into psum copy)
            psy0 = ap.tile([128, 257], F32, tag="psy", bufs=2, name=f"py{b}_0")
            psy1 = ap.tile([128, 257], F32, tag="psy", bufs=2, name=f"py{b}_1")
            for h, psy in ((0, psy0), (1, psy1)):
                for dc in range(2):
                    nc.tensor.matmul(
                        out=psy[:, :],
                        lhsT=rT[:, dc, h * 128:(h + 1) * 128],
                        rhs=w4[:, dc, 0:257],
                        start=(dc == 0), stop=(dc == 1))
            prev = (b, psy0, psy1)
        l4_tail(*prev)

        # ---- L4 stats + AR4 (exact: z4sb already carries bias) -------------
        arin4 = small.tile([128, 4], F32, tag="arin4")
        z4_stats(NG - 1)
        for h in range(2):
            nc.vector.tensor_reduce(out=arin4[:, 2 * h + 1:2 * h + 2],
                                    in_=q4[:, h, :],
                                    axis=mybir.AxisListType.XY, op=AL.add)
            nc.vector.tensor_reduce(out=arin4[:, 2 * h:2 * h + 1],
                                    in_=z4sb[:, :, h, 256:257],
                                    axis=mybir.AxisListType.XY, op=AL.add)
        artot4 = emit_allreduce(4, arin4, 4)
        s4, tsh4 = bn_finalize(3, artot4, exact_q=True, make_bst=False)

        # ---- final BN+relu: one ACT op per (b, h), then store ---------------
        for b in range(B_LOC):
            orl = stage.tile([128, 2, 256], BF16, tag="orl", name=f"or{b}")
            for h in range(2):
                nc.scalar.activation(out=orl[:, h, :], in_=z4sb[:, b, h, 0:256],
                                     func=ACT.Relu, scale=s4[:, h:h + 1],
                                     bias=tsh4[:, h:h + 1])
            nc.sync.dma_start(out=out_r[:, b, :, :], in_=orl[:])

        ap_cm.__exit__(None, None, None)

    _legalize_waits(nc)
    return nc


_CACHE = {}


def _prep_core_inputs(inputs):
    bf = ml_dtypes.bfloat16
    W = [inputs["W1"], inputs["W2"], inputs["W3"], inputs["W4"]]
    bs = [inputs["b1"], inputs["b2"], inputs["b3"], inputs["b4"]]
    gamma, beta = inputs["gamma"], inputs["beta"]

    # w123 cols: [0:256]=W1^T | 256+l = wsum_l (sum_e W_l) | [259:515]=W2^T
    # | [515:771]=W3^T
    w123 = np.zeros((128, 2, 771), dtype=bf)
    col0 = (0, 259, 515)
    for c in range(2):
        for l in range(3):
            w123[:, c, col0[l]:col0[l] + 256] = \
                W[l][:, c * 128:(c + 1) * 128].T.astype(bf)
            ws = W[l].astype(np.float64).sum(axis=0).astype(np.float32)
            w123[:, c, 256 + l] = ws[c * 128:(c + 1) * 128].astype(bf)
    w4 = np.zeros((128, 2, 257), dtype=bf)
    ws4 = W[3].astype(np.float64).sum(axis=0).astype(np.float32)
    for c in range(2):
        w4[:, c, 0:256] = W[3][:, c * 128:(c + 1) * 128].T.astype(bf)
        w4[:, c, 256] = ws4[c * 128:(c + 1) * 128].astype(bf)
    bb = np.broadcast_to(np.stack(bs, 0)[None], (128, 4, 256)).astype(bf)
    bb = np.ascontiguousarray(bb)
    gb = np.zeros((128, 2, 2), dtype=np.float32)
    for h in range(2):
        gb[:, h, 0] = gamma[h * 128:(h + 1) * 128]
        gb[:, h, 1] = beta[h * 128:(h + 1) * 128]
    hc = np.zeros((128, 8), dtype=np.float32)
    for l in range(4):
        hc[:, l] = bs[l].mean(dtype=np.float64)
    for l in range(3):
        hc[:, 4 + l] = (bs[l].astype(np.float64) ** 2).mean() - \
            bs[l].mean(dtype=np.float64) ** 2
    hc[:, 7] = (bs[3].astype(np.float64) ** 2).mean()
    return w123, w4, bb, gb, hc


def kernel(**inputs):
    _install_profile_shim()
    from concourse.bass_utils import run_bass_kernel_spmd

    if "nc" not in _CACHE:
        _CACHE["nc"] = build_program()
    nc = _CACHE["nc"]

    x = np.asarray(inputs["x"], dtype=np.float32)
    w123, w4, bb, gb, hc = _prep_core_inputs(
        {k: np.asarray(v) for k, v in inputs.items()})

    bf = ml_dtypes.bfloat16
    in_maps = []
    for i in range(NCORES):
        xs = x[i * B_LOC:(i + 1) * B_LOC].reshape(T, 256)
        # xT[p, dc, t] = xs[t, dc*128 + p]
        xTh = np.ascontiguousarray(
            xs.T.reshape(2, 128, T).transpose(1, 0, 2)).astype(bf)
        in_maps.append({"xT": xTh, "w123": w123, "w4": w4, "bb": bb,
                        "gb": gb, "hc": hc})

    trace = _CACHE.get("trace", False)
    res = run_bass_kernel_spmd(nc, in_maps, list(range(NCORES)), trace=trace)
    _CACHE["last_result"] = res

    out = np.empty((256, 256, 256), dtype=np.float32)
    for i in range(NCORES):
        out[i * B_LOC:(i + 1) * B_LOC] = np.asarray(
            res.results[i]["out"], dtype=np.float32).reshape(B_LOC, 256, 256)
    return out
